# revision 2
# baseline (speedup 1.0000x reference)
"""DilateAttention (kernel=9, dilation=3, hd=32) on 8 NeuronCores via Bass/Tile.

Inputs  q,k,v: [4, 512, 1, 4096] f32  (B, d, 1, L); d = 16 heads x 32.
Output        [4, 1, 4096, 512] f32  (heads concatenated per token).

Approach (v2): the dilation-3 attention graph splits into 3 independent
interleaved chains (positions mod 3), each an ordinary sliding-window
attention with window 9 and dilation 1.  The host deinterleaves q/k/v into a
"stored" sequence: [zeros 4][chain0][zeros 8][chain1][zeros 8][chain2][zeros]
of width W=4152 (8-gaps stop windows crossing chains; zero keys give
score 0 -> exp(0)=1, exactly nn.Unfold's zero-pad softmax semantics), casts
to bf16, and pre-transposes V into per-block slabs with a ones column (the
softmax denominator falls out of the PV matmul).

Device tiling: query blocks of BQ=56 read keys [t0-4, t0+60) = 64 wide.  Two
consecutive blocks are stacked on PSUM partitions (64+64=128), so every
elementwise op (exp on ACT, band mask on DVE/Pool, normalize) processes two
queries per 128-partition column - half the column count of a flat layout.
matmul tile_position handles the partition-offset outputs:
  mm1: lhsT=k[32hd, 64keys] rhs=q[32hd, 56] -> A[64s:64s+64, 56j:56j+56]
  PV : lhsT=P[64s:64s+64, 56]  rhs=vt[64s:64s+64, 33] -> C[64s:64s+56, 33j..]
C column 32 of each 33-slot is the denominator (ones column of vt);
reciprocal + broadcast multiply normalizes straight into the bf16 staging
tile, one output DMA per head pair.

Everything on the wire is bf16 (host casts); there are no on-device casts,
copies, or transposes.  Per core: 8 (b,h) pairs = 4 head-pairs; per pair 3
input DMAs (k, q, vt - split in two column chunks for startup overlap) and
1 output DMA.
"""

import os

import numpy as np

# Use the deterministic ASAP tile scheduler: the legacy CoreSim-based
# scheduler reorders instructions according to its own (slower-DMA) timing
# model, which head-of-line blocks the PE queue in the real timeline.  ASAP
# preserves emission order, which is software-pipelined below.
os.environ.setdefault("TILE_SCHEDULER", "asap")

import concourse.bacc as bacc
import concourse.bass as bass
import concourse.mybir as mybir
from concourse.tile import TileContext

B, D, L = 4, 512, 4096
HD = 32
NHEAD = D // HD          # 16
NCORES = 8
BH_PER_CORE = (B * NHEAD) // NCORES   # 8
NPAIR = BH_PER_CORE // 2              # 4 head-pairs per core

# stored (deinterleaved) layout
CLEN = [1366, 1365, 1365]            # chain lengths (L mod 3 == 1)
COFF = [4, 1378, 2751]               # chain start columns in stored coords
W = 4152                             # stored width (4 + 4144 + 4)
BQ = 56                              # queries per block
KK = 64                              # keys per block (BQ + 8)
NB = 74                              # blocks (74*56 = 4144 >= 4120)
NJ = NB // 2                         # 37 stacked block-pairs
GROUPS = [8, 8, 8, 8, 5]             # block-pairs per PSUM group (sum 37)
GMAX = max(GROUPS)
SCALE = float(HD) ** -0.5

# input-DMA column chunks (split after group 1 = block-pair 16 = block 32)
KSPLIT = 1688        # k cols [0,1688) cover blocks 0..29's slabs
QSPLIT = 1684        # q cols [0,1684) cover blocks 0..29's queries
VSPLIT = 33 * 30     # vt slots 0..29 (= block-pairs 0..14, both heads)

F32 = mybir.dt.float32
BF16 = mybir.dt.bfloat16

LABELS = {}  # instruction name -> semantic label (debug aid)


def _lab(inst, label):
    try:
        LABELS[inst.ins.name] = label
    except Exception:
        try:
            LABELS[inst.name] = label
        except Exception:
            pass
    return inst

# Units: one per (pair, biggroup, head); biggroups pack 16 (or 5) block-pair
# slots into 2-bank PSUM tiles (A and C are [128, 1024] f32, slots j<8 in
# bank 0, j>=8 at column 512+), so exp/band/recip/normalize run as one
# instruction per unit over multi-level access patterns.
G2 = [15, 15, 7]
# which full units run their band multiply on DVE (rest on Pool);
# stragglers (g==2) always on DVE.  Pool mult runs at 0.42 efficiency, DVE
# at 2x bf16, so the split is ~11 Pool fulls vs 5 DVE fulls + 8 stragglers.
DVE_BAND_UNITS = {(0, 0, 1), (1, 0, 1), (2, 0, 1), (3, 0, 1), (2, 1, 1)}


def _band_np():
    # band01[64s+u, 56j+t] = 1 iff key (t0-4+u) is a tap of query (t0+t):
    # u-t in [0, 8]; else 0.
    u = np.arange(KK)[:, None]
    t = np.arange(BQ)[None, :]
    b = ((u - t >= 0) & (u - t <= 8)).astype(np.float32)
    return np.tile(b, (2, 8))                              # [128, 448]


def _build_program():
    import ml_dtypes

    nc = bacc.Bacc(None, target_bir_lowering=False)
    k2 = nc.dram_tensor("k2", [NPAIR, 64, W], BF16, kind="ExternalInput")
    q2 = nc.dram_tensor("q2", [NPAIR, 64, W], BF16, kind="ExternalInput")
    vt = nc.dram_tensor("vt", [NPAIR, 128, 33 * 2 * NJ], BF16, kind="ExternalInput")
    outb = nc.dram_tensor("outb", [NPAIR, 128, 32 * 2 * NJ], BF16,
                          kind="ExternalOutput")

    band_dram = nc.inline_tensor(
        _band_np().astype(ml_dtypes.bfloat16), name="band01"
    )

    def ap3(base, *dims):
        return bass.AP(tensor=base.tensor, offset=base.offset,
                       ap=[base.ap[0]] + [list(d) for d in dims])

    with TileContext(nc) as tc:
        from contextlib import ExitStack

        eng = {"dve": nc.vector, "pool": nc.gpsimd}

        with ExitStack() as ctx:
            persist = ctx.enter_context(tc.tile_pool(name="persist", bufs=1))
            NSET = NPAIR  # one set per pair: no cross-pair reuse deps
            k_sb = [persist.tile([64, W], BF16, name=f"k{s}", tag=f"k{s}")
                    for s in range(NSET)]
            q_sb = [persist.tile([64, W], BF16, name=f"q{s}", tag=f"q{s}")
                    for s in range(NSET)]
            vt_sb = [persist.tile([128, 33 * 2 * NJ], BF16, name=f"v{s}",
                                  tag=f"v{s}") for s in range(NSET)]
            stage = [persist.tile([128, 32 * 2 * NJ], BF16, name=f"st{s}",
                                  tag=f"st{s}") for s in range(NSET)]
            band_sb = persist.tile([128, 448], BF16, name="band_sb",
                                   tag="band_sb")

            psA = ctx.enter_context(tc.tile_pool(name="psA", bufs=3, space="PSUM"))
            psC = ctx.enter_context(tc.tile_pool(name="psC", bufs=2, space="PSUM"))
            spP = ctx.enter_context(tc.tile_pool(name="spP", bufs=7))
            spR = ctx.enter_context(tc.tile_pool(name="spR", bufs=4))

            # all input DMAs up front (SP queue), chunked for startup overlap;
            # per-pair A then B so pair p is complete before pair p+1 starts
            # two column-chunks per tensor per pair, all on the SP queue
            # (one shared monotonic semaphore, so multi-chunk reads coalesce
            # into a single wait and no ldweights absorbers are needed).
            # Chunking halves the per-instruction transfer time in the Tile
            # scheduler's internal (slow) DMA model, so each pair's compute
            # gets scheduled interleaved with the previous pair's tail
            # instead of strictly after it.
            for p in range(NPAIR):
                _lab(nc.sync.dma_start(k_sb[p][:, 0:KSPLIT], k2[p, :, 0:KSPLIT]), f"dma_kA{p}")
                _lab(nc.sync.dma_start(q_sb[p][:, 0:QSPLIT], q2[p, :, 0:QSPLIT]), f"dma_qA{p}")
                if p == 0:
                    _lab(nc.sync.dma_start(band_sb[:, :], band_dram[:, :]),
                         "dma_band")
                _lab(nc.sync.dma_start(vt_sb[p][:, 0:VSPLIT], vt[p, :, 0:VSPLIT]), f"dma_vA{p}")
                _lab(nc.sync.dma_start(k_sb[p][:, KSPLIT:W], k2[p, :, KSPLIT:W]), f"dma_kB{p}")
                _lab(nc.sync.dma_start(q_sb[p][:, QSPLIT:W], q2[p, :, QSPLIT:W]), f"dma_qB{p}")
                _lab(nc.sync.dma_start(vt_sb[p][:, VSPLIT:], vt[p, :, VSPLIT:]), f"dma_vB{p}")

            # PE p-state warmup: the cost model runs matmuls 2-3.7x slower
            # until the PE has been continuously busy for 3us.  Chew through
            # that ramp on garbage data (stage[0] is not written until much
            # later, so these have no upstream deps) while the input DMAs
            # stream in.
            for i in range(0):
                Cd = psC.tile([128, 512], F32, name="warm", tag="C")
                nc.tensor.matmul(
                    Cd[0:64, 0:64],
                    stage[0][0:32, 0:64],
                    stage[0][0:32, 64:128],
                    start=True, stop=True,
                )

            # within a pair: h0's units (incl. its small straggler) first,
            # so the h0 out-DMA fires mid-pair; the pair (and the whole
            # schedule) drains on a small straggler unit instead of a full
            # group.
            UORDER = [(0, 0), (2, 0), (0, 1), (1, 0), (1, 1), (2, 1)]
            units = [(p, g, h) for p in range(NPAIR) for (g, h) in UORDER]
            state = {}  # unit -> (P tile, C tile, r tile, gsize)

            def emit_head(u):
                """mm1 batch -> A; exp -> P; band mult on P halves."""
                p, g, h = u
                gsize = G2[g]
                nb = (gsize + 7) // 8
                A = psA.tile([128, 1024], F32, name="A", tag="A")
                for j in range(gsize):
                    jg = 15 * g + j
                    acol = 512 * (j // 8) + 56 * (j % 8)
                    for st in range(2):
                        b = 2 * jg + st
                        _lab(nc.tensor.matmul(
                            A[64 * st : 64 * st + 64, acol : acol + 56],
                            k_sb[p][32 * h : 32 * h + 32, BQ * b : BQ * b + KK],
                            q_sb[p][32 * h : 32 * h + 32,
                                    4 + BQ * b : 4 + BQ * b + BQ],
                            start=True, stop=True,
                        ), f"mm1_{p}{g}{h}j{j}s{st}")
                P = spP.tile([128, 896], BF16, name="P", tag="P")
                if nb == 2:
                    # one exp over both banks (the 56-col tail of bank 1
                    # beyond slot 14 is stale garbage, never read)
                    in_ap = ap3(A[:, :], (512, 2), (1, 448))
                    out_ap = ap3(P[:, :], (448, 2), (1, 448))
                else:
                    in_ap = ap3(A[:, :], (1, 56 * gsize))
                    out_ap = ap3(P[:, :], (1, 56 * gsize))
                _lab(nc.scalar.activation(
                    out_ap, in_ap, mybir.ActivationFunctionType.Exp,
                    bias=0.0, scale=SCALE,
                ), f"exp_{p}{g}{h}")
                if nb == 2:
                    # bank-split: Pool (0.42-efficiency mult, otherwise idle)
                    # takes bank 0, DVE (2x bf16) bank 1; PV slots in each
                    # bank depend only on their half.
                    _lab(eng["pool"].tensor_tensor(
                        P[:, 0:448], P[:, 0:448], band_sb[:, :],
                        op=mybir.AluOpType.mult
                    ), f"band_{p}{g}{h}b0")
                    _lab(eng["dve"].tensor_tensor(
                        P[:, 448:840], P[:, 448:840], band_sb[:, 0:392],
                        op=mybir.AluOpType.mult
                    ), f"band_{p}{g}{h}b1")
                else:
                    _lab(eng["dve" if h == 0 else "pool"].tensor_tensor(
                        P[:, 0 : 56 * gsize], P[:, 0 : 56 * gsize],
                        band_sb[:, 0 : 56 * gsize], op=mybir.AluOpType.mult
                    ), f"band_{p}{g}{h}")
                state[u] = (A, P, gsize)

            def emit_tail(u):
                """PV batch -> C (one bank); fused divide-normalize -> stage;
                out-DMA at pair end."""
                p, g, h = u
                A, P, gsize = state.pop(u)
                C = psC.tile([128, 512], F32, name="C", tag="C")
                for j in range(gsize):
                    jg = 15 * g + j
                    pcol = 448 * (j // 8) + 56 * (j % 8)
                    for st in range(2):
                        _lab(nc.tensor.matmul(
                            C[64 * st : 64 * st + 56, 33 * j : 33 * j + 33],
                            P[64 * st : 64 * st + 64, pcol : pcol + 56],
                            vt_sb[p][64 * st : 64 * st + 64,
                                     33 * (2 * jg + h) : 33 * (2 * jg + h) + 33],
                            start=True, stop=True,
                        ), f"pv_{p}{g}{h}j{j}s{st}")
                r = spR.tile([128, 16], F32, name="r", tag="r")
                _lab(nc.vector.reciprocal_approx_fast(
                    out=ap3(r[:, :], (1, gsize)),
                    in_=ap3(C[:, 32:33], (33, gsize)),
                ), f"recip_{p}{g}{h}")
                c0 = 32 * NJ * h + 32 * 15 * g
                st_ap = ap3(stage[p][:, c0 : c0 + 1], (32, gsize), (1, 32))
                num_ap = ap3(C[:, :], (33, gsize), (1, 32))
                r_ap = ap3(r[:, :], (1, gsize), (0, 32))
                _lab(nc.vector.tensor_tensor(
                    st_ap, num_ap, r_ap, op=mybir.AluOpType.mult
                ), f"norm_{p}{g}{h}")
                if (g, h) in ((1, 0), (2, 1)):  # last unit of this head
                    o0 = 32 * NJ * h
                    for st in range(2):
                        _lab(nc.sync.dma_start(
                            outb[p, 64 * st : 64 * st + 56, o0 : o0 + 32 * NJ],
                            stage[p][64 * st : 64 * st + 56, o0 : o0 + 32 * NJ]),
                            f"dma_out{p}h{h}s{st}")

            # Software-pipelined emission (ASAP scheduler preserves this
            # order).  Per iteration i:
            #   PE:   mm1(i), pv(i-SKEW)
            #   ACT:  exp(i)
            #   Pool: band[0:448](i)      DVE: band[448:840](i)
            #   DVE:  recip(i-SKEW), norm(i-SKEW)
            #   SP:   out-DMA when unit i-SKEW closes a (pair, head)
            # so the PE never waits on the exp/band chain of the unit whose
            # PV batch it is about to run.
            SKEW = 3
            pend = []
            for u in units:
                emit_head(u)
                pend.append(u)
                if len(pend) > SKEW:
                    emit_tail(pend.pop(0))
            for v in pend:
                emit_tail(v)
    nc.finalize()
    return nc


_CACHE = {}


def _get_program():
    if "nc" not in _CACHE:
        _CACHE["nc"] = _build_program()
    return _CACHE["nc"]


def _store(x):
    """Deinterleave [BH, HD, L] -> stored [BH, HD, W] (zero pads/gaps)."""
    S = np.zeros((x.shape[0], x.shape[1], W), dtype=np.float32)
    for r in range(3):
        S[:, :, COFF[r] : COFF[r] + CLEN[r]] = x[:, :, r::3]
    return S


def make_in_maps(q, k, v):
    """Shard + pack FULL inputs into per-core input maps (host-side data
    movement only)."""
    import ml_dtypes

    q = np.ascontiguousarray(np.asarray(q), dtype=np.float32)
    k = np.ascontiguousarray(np.asarray(k), dtype=np.float32)
    v = np.ascontiguousarray(np.asarray(v), dtype=np.float32)
    Sq = _store(q.reshape(B * NHEAD, HD, L))
    Sk = _store(k.reshape(B * NHEAD, HD, L))
    Sv = _store(v.reshape(B * NHEAD, HD, L))

    swv = np.lib.stride_tricks.sliding_window_view  # (arr, KK, axis)
    in_maps = []
    for c in range(NCORES):
        base = c * BH_PER_CORE
        k2 = np.empty((NPAIR, 64, W), dtype=ml_dtypes.bfloat16)
        q2 = np.empty((NPAIR, 64, W), dtype=ml_dtypes.bfloat16)
        vtm = np.empty((NPAIR, 128, 33 * 2 * NJ), dtype=ml_dtypes.bfloat16)
        for p in range(NPAIR):
            i, j = base + 2 * p, base + 2 * p + 1
            k2[p, 0:32] = Sk[i]
            k2[p, 32:64] = Sk[j]
            q2[p, 0:32] = Sq[i]
            q2[p, 32:64] = Sq[j]
            # vt rows 64s+u, cols 33*(2*jg+h)+c ; ones at c=32
            arr = np.ones((2, 64, NJ, 2, 33), dtype=np.float32)
            for h, bh in enumerate((i, j)):
                sl = swv(Sv[bh], KK, axis=1)[:, :: BQ, :][:, :NB]  # [32,74,64]
                slT = sl.transpose(1, 2, 0).reshape(NJ, 2, KK, HD)
                # arr[s, u, jg, h, 0:32] = slT[jg, s, u, :]
                arr[:, :, :, h, 0:32] = slT.transpose(1, 2, 0, 3)
            vtm[p] = arr.reshape(128, 33 * 2 * NJ)
        in_maps.append({"k2": k2, "q2": q2, "vt": vtm})
    return in_maps


# stored-column gather index: position n -> stored col COFF[n%3] + n//3,
# minus 4 (stage covers stored cols [4, 4+74*56))
_N = np.arange(L)
_IDX = np.array([COFF[n % 3] + n // 3 - 4 for n in range(L)])


def assemble_output(results):
    """results: list of 8 per-core dicts with 'outb' [NPAIR, 128, 2368]."""
    full = np.empty((B * NHEAD, L, HD), dtype=np.float32)
    for c in range(NCORES):
        arr = np.asarray(results[c]["outb"]).astype(np.float32)
        for p in range(NPAIR):
            for h in range(2):
                bh = c * BH_PER_CORE + 2 * p + h
                blk = arr[p, :, 32 * NJ * h : 32 * NJ * (h + 1)]
                blk = blk.reshape(2, 64, NJ, HD)[:, 0:BQ]   # [2, 56, 37, 32]
                so = blk.transpose(2, 0, 1, 3).reshape(NB * BQ, HD)
                full[bh] = so[_IDX]
    out = full.reshape(B, NHEAD, L, HD).transpose(0, 2, 1, 3)
    return np.ascontiguousarray(out.reshape(B, 1, L, D))


def kernel(q, k, v):
    from concourse.bass_utils import run_bass_kernel_spmd

    in_maps = make_in_maps(q, k, v)
    nc = _get_program()
    res = run_bass_kernel_spmd(nc, in_maps, core_ids=list(range(NCORES)))
    return assemble_output(res.results)


if __name__ == "__main__":
    rng = np.random.default_rng(0)
    q = rng.standard_normal((B, D, 1, L), dtype=np.float32)
    k = rng.standard_normal((B, D, 1, L), dtype=np.float32)
    v = rng.standard_normal((B, D, 1, L), dtype=np.float32)
    o = kernel(q=q, k=k, v=v)
    print("out", o.shape, o.dtype, float(np.abs(o).max()))


# revision 4
# speedup vs baseline: 1.0150x; 1.0150x over previous
"""DilateAttention (kernel=9, dilation=3, hd=32) on 8 NeuronCores via Bass/Tile.

Inputs  q,k,v: [4, 512, 1, 4096] f32  (B, d, 1, L); d = 16 heads x 32.
Output        [4, 1, 4096, 512] f32  (heads concatenated per token).

The dilation-3 attention graph splits into 3 independent interleaved chains
(positions mod 3), each an ordinary sliding-window attention with window 9
and dilation 1.  The host deinterleaves q/k/v into a "stored" sequence
[zeros 4][chain0][zeros 8][chain1][zeros 8][chain2][zeros] of width W=4152
(8-gaps stop windows crossing chains; zero keys score 0 -> exp(0)=1, exactly
nn.Unfold's zero-pad softmax semantics), casts everything to bf16, and
pre-transposes V into per-block key slabs with a ones column (the softmax
denominator falls out of the PV matmul).  The host reassembles and casts the
bf16 output back to f32.

Device tiling: query blocks of BQ=56 read keys [t0-4, t0+60) = 64 wide; two
consecutive blocks stack on PSUM partitions (64+64=128), halving every
elementwise op's column count.  15 block-pair slots form one "unit" whose
score tile A spans 2 PSUM banks (slots j<8 bank 0, j>=8 at column 512+):
  mm1: lhsT=k[32hd, 64keys] rhs=q[32hd, 56] -> A[64s:64s+64, acol(j)+56]
  exp: one ACT instruction over both banks (3-level AP), bf16 P in SBUF
  band mask: 0/1 multiply, bank 0 on Pool, bank 1 on DVE (2x bf16 mode)
  PV : lhsT=P[64s:64s+64, 56]  rhs=vt[64s:64s+64, 33] -> C[64s:64s+56, 33j]
  recip (DVE) of the ones-column denominators + broadcast multiply
  normalizes straight into the bf16 staging tile; one output DMA per
  (pair, head) half.
Per core: 8 (b,h) = 4 head-pairs x 2 heads x 3 groups = 24 units.

Scheduling: TILE_SCHEDULER=asap preserves emission order (the legacy
CoreSim-based scheduler reorders around its own slower DMA model and
head-of-line blocks the in-order PE queue).  The emission is software-
pipelined with skew 3 (PV/recip/normalize of unit u follow the mm1 batch of
unit u+3), PSUM pools are psA bufs=3 / psC bufs=2, and input DMAs are
column-chunked so the first unit starts after ~1.3MB instead of the full
pair.  All input DMAs ride one SP queue whose monotonic semaphore coalesces
multi-chunk dependencies.

TimelineSim: 42217 ns/core (baseline flat f32 implementation: 82760 ns).
"""

import os

import numpy as np

# Use the deterministic ASAP tile scheduler: the legacy CoreSim-based
# scheduler reorders instructions according to its own (slower-DMA) timing
# model, which head-of-line blocks the PE queue in the real timeline.  ASAP
# preserves emission order, which is software-pipelined below.
os.environ.setdefault("TILE_SCHEDULER", "asap")

import concourse.bacc as bacc
import concourse.bass as bass
import concourse.mybir as mybir
from concourse.tile import TileContext

B, D, L = 4, 512, 4096
HD = 32
NHEAD = D // HD          # 16
NCORES = 8
BH_PER_CORE = (B * NHEAD) // NCORES   # 8
NPAIR = BH_PER_CORE // 2              # 4 head-pairs per core

# stored (deinterleaved) layout
CLEN = [1366, 1365, 1365]            # chain lengths (L mod 3 == 1)
COFF = [4, 1378, 2751]               # chain start columns in stored coords
W = 4152                             # stored width (4 + 4144 + 4)
BQ = 56                              # queries per block
KK = 64                              # keys per block (BQ + 8)
NB = 74                              # blocks (74*56 = 4144 >= 4120)
NJ = NB // 2                         # 37 stacked block-pairs
SCALE = float(HD) ** -0.5

# input-DMA column chunks (split after group 1 = block-pair 16 = block 32)
KSPLIT = 1688        # k cols [0,1688) cover blocks 0..29's slabs
QSPLIT = 1684        # q cols [0,1684) cover blocks 0..29's queries
VSPLIT = 33 * 30     # vt slots 0..29 (= block-pairs 0..14, both heads)

F32 = mybir.dt.float32
BF16 = mybir.dt.bfloat16

LABELS = {}  # instruction name -> semantic label (debug aid)


def _lab(inst, label):
    try:
        LABELS[inst.ins.name] = label
    except Exception:
        try:
            LABELS[inst.name] = label
        except Exception:
            pass
    return inst

# Units: one per (pair, biggroup, head); biggroups pack 16 (or 5) block-pair
# slots into 2-bank PSUM tiles (A and C are [128, 1024] f32, slots j<8 in
# bank 0, j>=8 at column 512+), so exp/band/recip/normalize run as one
# instruction per unit over multi-level access patterns.
G2 = [15, 15, 7]
# per-pair unit order: h0 first (incl. its straggler) so the h0 out-DMA
# fires mid-pair; pair 0 pushes stragglers last since they need chunk B
UORDER = [(0, 0), (2, 0), (0, 1), (1, 0), (1, 1), (2, 1)]


def _band_np():
    # band01[64s+u, 56j+t] = 1 iff key (t0-4+u) is a tap of query (t0+t):
    # u-t in [0, 8]; else 0.
    u = np.arange(KK)[:, None]
    t = np.arange(BQ)[None, :]
    b = ((u - t >= 0) & (u - t <= 8)).astype(np.float32)
    return np.tile(b, (2, 8))                              # [128, 448]


def _build_program():
    import ml_dtypes

    nc = bacc.Bacc(None, target_bir_lowering=False)
    k2 = nc.dram_tensor("k2", [NPAIR, 64, W], BF16, kind="ExternalInput")
    q2 = nc.dram_tensor("q2", [NPAIR, 64, W], BF16, kind="ExternalInput")
    vt = nc.dram_tensor("vt", [NPAIR, 128, 33 * 2 * NJ], BF16, kind="ExternalInput")
    outb = nc.dram_tensor("outb", [NPAIR, 128, 32 * 2 * NJ], BF16,
                          kind="ExternalOutput")

    band_dram = nc.inline_tensor(
        _band_np().astype(ml_dtypes.bfloat16), name="band01"
    )

    def ap3(base, *dims):
        return bass.AP(tensor=base.tensor, offset=base.offset,
                       ap=[base.ap[0]] + [list(d) for d in dims])

    with TileContext(nc) as tc:
        from contextlib import ExitStack

        eng = {"dve": nc.vector, "pool": nc.gpsimd}

        with ExitStack() as ctx:
            persist = ctx.enter_context(tc.tile_pool(name="persist", bufs=1))
            NSET = NPAIR  # one set per pair: no cross-pair reuse deps
            k_sb = [persist.tile([64, W], BF16, name=f"k{s}", tag=f"k{s}")
                    for s in range(NSET)]
            q_sb = [persist.tile([64, W], BF16, name=f"q{s}", tag=f"q{s}")
                    for s in range(NSET)]
            vt_sb = [persist.tile([128, 33 * 2 * NJ], BF16, name=f"v{s}",
                                  tag=f"v{s}") for s in range(NSET)]
            stage = [persist.tile([128, 32 * 2 * NJ], BF16, name=f"st{s}",
                                  tag=f"st{s}") for s in range(NSET)]
            band_sb = persist.tile([128, 448], BF16, name="band_sb",
                                   tag="band_sb")

            psA = ctx.enter_context(tc.tile_pool(name="psA", bufs=3, space="PSUM"))
            psC = ctx.enter_context(tc.tile_pool(name="psC", bufs=2, space="PSUM"))
            spP = ctx.enter_context(tc.tile_pool(name="spP", bufs=7))
            spR = ctx.enter_context(tc.tile_pool(name="spR", bufs=4))

            # all input DMAs up front (SP queue), chunked for startup overlap;
            # per-pair A then B so pair p is complete before pair p+1 starts
            # two column-chunks per tensor per pair, all on the SP queue
            # (one shared monotonic semaphore, so multi-chunk reads coalesce
            # into a single wait and no ldweights absorbers are needed).
            # Chunking halves the per-instruction transfer time in the Tile
            # scheduler's internal (slow) DMA model, so each pair's compute
            # gets scheduled interleaved with the previous pair's tail
            # instead of strictly after it.
            for p in range(NPAIR):
                _lab(nc.sync.dma_start(k_sb[p][:, 0:KSPLIT], k2[p, :, 0:KSPLIT]), f"dma_kA{p}")
                _lab(nc.sync.dma_start(q_sb[p][:, 0:QSPLIT], q2[p, :, 0:QSPLIT]), f"dma_qA{p}")
                if p == 0:
                    _lab(nc.sync.dma_start(band_sb[:, :], band_dram[:, :]),
                         "dma_band")
                _lab(nc.sync.dma_start(vt_sb[p][:, 0:VSPLIT], vt[p, :, 0:VSPLIT]), f"dma_vA{p}")
                _lab(nc.sync.dma_start(k_sb[p][:, KSPLIT:W], k2[p, :, KSPLIT:W]), f"dma_kB{p}")
                _lab(nc.sync.dma_start(q_sb[p][:, QSPLIT:W], q2[p, :, QSPLIT:W]), f"dma_qB{p}")
                _lab(nc.sync.dma_start(vt_sb[p][:, VSPLIT:], vt[p, :, VSPLIT:]), f"dma_vB{p}")

            # PE p-state warmup: the cost model runs matmuls 2-3.7x slower
            # until the PE has been continuously busy for 3us.  Chew through
            # that ramp on garbage data (stage[0] is not written until much
            # later, so these have no upstream deps) while the input DMAs
            # stream in.
            for i in range(0):
                Cd = psC.tile([128, 512], F32, name="warm", tag="C")
                nc.tensor.matmul(
                    Cd[0:64, 0:64],
                    stage[0][0:32, 0:64],
                    stage[0][0:32, 64:128],
                    start=True, stop=True,
                )

            # within a pair: h0's units (incl. its small straggler) first,
            # so the h0 out-DMA fires mid-pair; the pair (and the whole
            # schedule) drains on a small straggler unit instead of a full
            # group.
            units = [(p, g, h) for p in range(NPAIR) for (g, h) in UORDER]
            state = {}  # unit -> (P tile, C tile, r tile, gsize)

            def emit_head(u):
                """mm1 batch -> A; exp -> P; band mult on P halves."""
                p, g, h = u
                gsize = G2[g]
                nb = (gsize + 7) // 8
                A = psA.tile([128, 1024], F32, name="A", tag="A")
                for j in range(gsize):
                    jg = 15 * g + j
                    acol = 512 * (j // 8) + 56 * (j % 8)
                    for st in range(2):
                        b = 2 * jg + st
                        _lab(nc.tensor.matmul(
                            A[64 * st : 64 * st + 64, acol : acol + 56],
                            k_sb[p][32 * h : 32 * h + 32, BQ * b : BQ * b + KK],
                            q_sb[p][32 * h : 32 * h + 32,
                                    4 + BQ * b : 4 + BQ * b + BQ],
                            start=True, stop=True,
                        ), f"mm1_{p}{g}{h}j{j}s{st}")
                P = spP.tile([128, 896], BF16, name="P", tag="P")
                if nb == 2:
                    # one exp over both banks (the 56-col tail of bank 1
                    # beyond slot 14 is stale garbage, never read)
                    in_ap = ap3(A[:, :], (512, 2), (1, 448))
                    out_ap = ap3(P[:, :], (448, 2), (1, 448))
                else:
                    in_ap = ap3(A[:, :], (1, 56 * gsize))
                    out_ap = ap3(P[:, :], (1, 56 * gsize))
                _lab(nc.scalar.activation(
                    out_ap, in_ap, mybir.ActivationFunctionType.Exp,
                    bias=0.0, scale=SCALE,
                ), f"exp_{p}{g}{h}")
                if nb == 2:
                    # bank-split: Pool (0.42-efficiency mult, otherwise idle)
                    # takes bank 0, DVE (2x bf16) bank 1; PV slots in each
                    # bank depend only on their half.
                    _lab(eng["pool"].tensor_tensor(
                        P[:, 0:448], P[:, 0:448], band_sb[:, :],
                        op=mybir.AluOpType.mult
                    ), f"band_{p}{g}{h}b0")
                    _lab(eng["dve"].tensor_tensor(
                        P[:, 448:840], P[:, 448:840], band_sb[:, 0:392],
                        op=mybir.AluOpType.mult
                    ), f"band_{p}{g}{h}b1")
                else:
                    _lab(eng["dve" if h == 0 else "pool"].tensor_tensor(
                        P[:, 0 : 56 * gsize], P[:, 0 : 56 * gsize],
                        band_sb[:, 0 : 56 * gsize], op=mybir.AluOpType.mult
                    ), f"band_{p}{g}{h}")
                state[u] = (A, P, gsize)

            def emit_tail(u):
                """PV batch -> C (one bank); fused divide-normalize -> stage;
                out-DMA at pair end."""
                p, g, h = u
                A, P, gsize = state.pop(u)
                C = psC.tile([128, 512], F32, name="C", tag="C")
                # bank-1 slots first: their band half runs on the fast DVE,
                # so those PV matmuls are ready ~700ns before bank 0's
                for j in list(range(8, gsize)) + list(range(min(8, gsize))):
                    jg = 15 * g + j
                    pcol = 448 * (j // 8) + 56 * (j % 8)
                    for st in range(2):
                        _lab(nc.tensor.matmul(
                            C[64 * st : 64 * st + 56, 33 * j : 33 * j + 33],
                            P[64 * st : 64 * st + 64, pcol : pcol + 56],
                            vt_sb[p][64 * st : 64 * st + 64,
                                     33 * (2 * jg + h) : 33 * (2 * jg + h) + 33],
                            start=True, stop=True,
                        ), f"pv_{p}{g}{h}j{j}s{st}")
                r = spR.tile([128, 16], F32, name="r", tag="r")
                _lab(nc.vector.reciprocal_approx_fast(
                    out=ap3(r[:, :], (1, gsize)),
                    in_=ap3(C[:, 32:33], (33, gsize)),
                ), f"recip_{p}{g}{h}")
                c0 = 32 * NJ * h + 32 * 15 * g
                st_ap = ap3(stage[p][:, c0 : c0 + 1], (32, gsize), (1, 32))
                num_ap = ap3(C[:, :], (33, gsize), (1, 32))
                r_ap = ap3(r[:, :], (1, gsize), (0, 32))
                _lab(nc.vector.tensor_tensor(
                    st_ap, num_ap, r_ap, op=mybir.AluOpType.mult
                ), f"norm_{p}{g}{h}")
                order = UORDER
                last_of_head = max(i for i, gh in enumerate(order) if gh[1] == h)
                if order.index((g, h)) == last_of_head:
                    o0 = 32 * NJ * h
                    if (p, h) == (NPAIR - 1, 1):
                        # the final out gates the program end through the
                        # full DMA-path latency; ship the bulk as soon as
                        # groups 0/1 are normalized and only the 7-slot
                        # straggler tail after the last normalize
                        _lab(nc.sync.dma_start(
                            outb[p, :, o0 : o0 + 960],
                            stage[p][:, o0 : o0 + 960]),
                            f"dma_out{p}h{h}a")
                        _lab(nc.sync.dma_start(
                            outb[p, :, o0 + 960 : o0 + 32 * NJ],
                            stage[p][:, o0 + 960 : o0 + 32 * NJ]),
                            f"dma_out{p}h{h}b")
                    else:
                        _lab(nc.sync.dma_start(
                            outb[p, :, o0 : o0 + 32 * NJ],
                            stage[p][:, o0 : o0 + 32 * NJ]),
                            f"dma_out{p}h{h}")

            # Software-pipelined emission (ASAP scheduler preserves this
            # order).  Per iteration i:
            #   PE:   mm1(i), pv(i-SKEW)
            #   ACT:  exp(i)
            #   Pool: band[0:448](i)      DVE: band[448:840](i)
            #   DVE:  recip(i-SKEW), norm(i-SKEW)
            #   SP:   out-DMA when unit i-SKEW closes a (pair, head)
            # so the PE never waits on the exp/band chain of the unit whose
            # PV batch it is about to run.
            SKEW = 3
            pend = []
            for i, u in enumerate(units):
                emit_head(u)
                pend.append(u)
                # shallower skew over the final units so the drain chain is
                # short when the schedule runs out of heads
                lim = SKEW if i < len(units) - 2 else 1
                while len(pend) > lim:
                    emit_tail(pend.pop(0))
            for v in pend:
                emit_tail(v)
    nc.finalize()
    return nc


_CACHE = {}


def _get_program():
    if "nc" not in _CACHE:
        _CACHE["nc"] = _build_program()
    return _CACHE["nc"]


def _store(x):
    """Deinterleave [BH, HD, L] -> stored [BH, HD, W] (zero pads/gaps)."""
    S = np.zeros((x.shape[0], x.shape[1], W), dtype=np.float32)
    for r in range(3):
        S[:, :, COFF[r] : COFF[r] + CLEN[r]] = x[:, :, r::3]
    return S


def make_in_maps(q, k, v):
    """Shard + pack FULL inputs into per-core input maps (host-side data
    movement only)."""
    import ml_dtypes

    q = np.ascontiguousarray(np.asarray(q), dtype=np.float32)
    k = np.ascontiguousarray(np.asarray(k), dtype=np.float32)
    v = np.ascontiguousarray(np.asarray(v), dtype=np.float32)
    Sq = _store(q.reshape(B * NHEAD, HD, L))
    Sk = _store(k.reshape(B * NHEAD, HD, L))
    Sv = _store(v.reshape(B * NHEAD, HD, L))

    swv = np.lib.stride_tricks.sliding_window_view  # (arr, KK, axis)
    in_maps = []
    for c in range(NCORES):
        base = c * BH_PER_CORE
        k2 = np.empty((NPAIR, 64, W), dtype=ml_dtypes.bfloat16)
        q2 = np.empty((NPAIR, 64, W), dtype=ml_dtypes.bfloat16)
        vtm = np.empty((NPAIR, 128, 33 * 2 * NJ), dtype=ml_dtypes.bfloat16)
        for p in range(NPAIR):
            i, j = base + 2 * p, base + 2 * p + 1
            k2[p, 0:32] = Sk[i]
            k2[p, 32:64] = Sk[j]
            q2[p, 0:32] = Sq[i]
            q2[p, 32:64] = Sq[j]
            # vt rows 64s+u, cols 33*(2*jg+h)+c ; ones at c=32
            arr = np.ones((2, 64, NJ, 2, 33), dtype=np.float32)
            for h, bh in enumerate((i, j)):
                sl = swv(Sv[bh], KK, axis=1)[:, :: BQ, :][:, :NB]  # [32,74,64]
                slT = sl.transpose(1, 2, 0).reshape(NJ, 2, KK, HD)
                # arr[s, u, jg, h, 0:32] = slT[jg, s, u, :]
                arr[:, :, :, h, 0:32] = slT.transpose(1, 2, 0, 3)
            vtm[p] = arr.reshape(128, 33 * 2 * NJ)
        in_maps.append({"k2": k2, "q2": q2, "vt": vtm})
    return in_maps


# stored-column gather index: position n -> stored col COFF[n%3] + n//3,
# minus 4 (stage covers stored cols [4, 4+74*56))
_N = np.arange(L)
_IDX = np.array([COFF[n % 3] + n // 3 - 4 for n in range(L)])


def assemble_output(results):
    """results: list of 8 per-core dicts with 'outb' [NPAIR, 128, 2368]."""
    full = np.empty((B * NHEAD, L, HD), dtype=np.float32)
    for c in range(NCORES):
        arr = np.asarray(results[c]["outb"]).astype(np.float32)
        for p in range(NPAIR):
            for h in range(2):
                bh = c * BH_PER_CORE + 2 * p + h
                blk = arr[p, :, 32 * NJ * h : 32 * NJ * (h + 1)]
                blk = blk.reshape(2, 64, NJ, HD)[:, 0:BQ]   # [2, 56, 37, 32]
                so = blk.transpose(2, 0, 1, 3).reshape(NB * BQ, HD)
                full[bh] = so[_IDX]
    out = full.reshape(B, NHEAD, L, HD).transpose(0, 2, 1, 3)
    return np.ascontiguousarray(out.reshape(B, 1, L, D))


def kernel(q, k, v):
    from concourse.bass_utils import run_bass_kernel_spmd

    in_maps = make_in_maps(q, k, v)
    nc = _get_program()
    res = run_bass_kernel_spmd(nc, in_maps, core_ids=list(range(NCORES)))
    return assemble_output(res.results)


if __name__ == "__main__":
    rng = np.random.default_rng(0)
    q = rng.standard_normal((B, D, 1, L), dtype=np.float32)
    k = rng.standard_normal((B, D, 1, L), dtype=np.float32)
    v = rng.standard_normal((B, D, 1, L), dtype=np.float32)
    o = kernel(q=q, k=k, v=v)
    print("out", o.shape, o.dtype, float(np.abs(o).max()))
